# revision 1
# baseline (speedup 1.0000x reference)
"""MoChA stable chunkwise attention (window w=16) on 8 Trainium2 NeuronCores.

The reference's stabilizing moving-max cancels algebraically:
    P[t] = exp(logits[t]);  S[u] = sum_{v=u-15..u} P[v]
    R[u] = emit[u]/S[u];    out[t] = P[t] * sum_{k=0..15} R[t+k]
Both width-16 window sums run on the TensorEngine as banded matmuls in a
transposed layout (partition = t mod 128, free = (block, chunk) columns);
cross-block window wrap is handled by corner matmuls accumulating in PSUM,
with zero-masked operands at row boundaries. The host pre-permutes all
inputs into device layout (plain contiguous DMA loads, no on-device
transposes) and un-permutes the output. Logits travel as fp16 value +
fp16 residual planes whose on-device sum restores fp32 accuracy.

Self-contained: only numpy + concourse (on PYTHONPATH) required.
"""

import numpy as np

import concourse.bass as bass
import concourse.tile as tile
import concourse.mybir as mybir
from concourse import bacc
from concourse.bass_utils import run_bass_kernel_spmd

F32 = mybir.dt.float32
F16 = mybir.dt.float16
ACTF = mybir.ActivationFunctionType

B, T = 64, 16384
NCORES = 8
RPC = B // NCORES        # 8 rows/core
NCH = 16                 # chunks per row
CH = 1024                # elems per chunk
NPART = 128
NBLK = CH // 128         # 8 blocks per chunk
W = 16                   # window
NF = RPC * T // 128      # 1024 layout-B columns


def make_consts():
    k = np.arange(128)[:, None]
    m = np.arange(128)[None, :]
    band0 = (m - k >= 0) & (m - k <= W - 1)            # S within-block
    corner = (k - m >= 128 - W + 1) & (k - m <= 127)   # S from prev block
    banda = (k - m >= 0) & (k - m <= W - 1)            # Z within-block
    cornera = (m - k >= 128 - W + 1) & (m - k <= 127)  # Z from next block
    return np.concatenate(
        [x.astype(np.float16) for x in (band0, corner, banda, cornera)],
        axis=1,
    )  # [128, 512]


def _perm(a):
    """[RPC, T] -> layout B [128, NF]: full host-side transpose, so device
    loads are plain contiguous [128 partitions x NF] DMAs (no xbar)."""
    return np.ascontiguousarray(
        a.reshape(RPC, NCH, NBLK, 128).transpose(3, 2, 0, 1).reshape(128, NF)
    )


def unperm_out(o):
    """[128, NF] layout B -> [RPC, T]."""
    return np.ascontiguousarray(
        o.reshape(128, NBLK, RPC, NCH)
        .transpose(2, 3, 1, 0)
        .reshape(RPC, T)
    )


def build_nc():
    nc = bacc.Bacc("TRN2", target_bir_lowering=False, debug=False,
                   num_devices=NCORES)
    hi_t = nc.dram_tensor("lg_hi", [NPART, NF], F16, kind="ExternalInput")
    lo_t = nc.dram_tensor("lg_lo", [NPART, NF], F16, kind="ExternalInput")
    em_t = nc.dram_tensor("em16", [NPART, NF], F16, kind="ExternalInput")
    kc_t = nc.dram_tensor("consts16", [NPART, 512], F16, kind="ExternalInput")
    out_t = nc.dram_tensor("out", [NPART, NF], F32, kind="ExternalOutput")

    H1 = slice(512, 1024)
    H0 = slice(0, 512)

    with tile.TileContext(nc) as tc:
        with (
            tc.tile_pool(name="sb", bufs=1) as sb,
            tc.tile_pool(name="ps", bufs=1, space="PSUM") as ps,
        ):
            kb = sb.tile([NPART, 512], F16, tag="kb")
            hi_b = sb.tile([NPART, CH], F16, tag="hi_b")
            lo_b = sb.tile([NPART, CH], F16, tag="lo_b")
            lg_b = sb.tile([NPART, CH], F32, tag="lg_b")
            e_b = sb.tile([NPART, CH], F16, tag="e_b")
            p_b = sb.tile([NPART, CH], F16, tag="p_b")
            rcp_b = sb.tile([NPART, CH], F32, tag="rcp_b")
            r_b = sb.tile([NPART, CH], F16, tag="r_b")
            o_b = sb.tile([NPART, CH], F32, tag="o_b")

            pz_b = sb.tile([NPART, 129], F16, tag="pz_b")
            rz_b = sb.tile([NPART, 129], F16, tag="rz_b")
            s_ps = ps.tile([NPART, CH], F32, tag="s")
            z_ps = ps.tile([NPART, CH], F32, tag="z")

            band0 = kb[:, 0:128]
            corner = kb[:, 128:256]
            banda = kb[:, 256:384]
            cornera = kb[:, 384:512]

            # ---- loads: all plain contiguous DMAs, spread over both HWDGE;
            # h1 planes first, h0 planes right behind, S-consts between,
            # Z-consts and emit last (needed latest) ----
            nc.sync.dma_start(
                hi_b[:, 512:1024], bass.AP(hi_t, 512, [[NF, NPART], [1, 512]]))
            nc.scalar.dma_start(
                lo_b[:, 512:1024], bass.AP(lo_t, 512, [[NF, NPART], [1, 512]]))
            nc.sync.dma_start(
                hi_b[:, 0:512], bass.AP(hi_t, 0, [[NF, NPART], [1, 512]]))
            nc.scalar.dma_start(
                lo_b[:, 0:512], bass.AP(lo_t, 0, [[NF, NPART], [1, 512]]))
            nc.sync.dma_start(kb[:, 0:256],
                              bass.AP(kc_t, 0, [[512, NPART], [1, 256]]))
            nc.scalar.dma_start(
                e_b[:, :], bass.AP(em_t, 0, [[NF, NPART], [1, NF]]))
            nc.sync.dma_start(kb[:, 256:512],
                              bass.AP(kc_t, 256, [[512, NPART], [1, 256]]))

            # ---- logits = hi + lo (fp32), exp -> fp16 P; h1 first (the S
            # block-0 corner reads block 7) ----
            for h in (H1, H0):
                nc.vector.tensor_add(lg_b[:, h], hi_b[:, h], lo_b[:, h])
                nc.scalar.activation(p_b[:, h], lg_b[:, h], ACTF.Exp)

            # masked wrap operand for S block 0: pz[:, j] = p_b[:, 896+j-1],
            # zeroed at j==0 and j%16==0 (row starts)
            nc.scalar.copy(pz_b[:, 1:128], p_b[:, 896:1023])
            nc.vector.memset(pz_b[:, 0:1], 0.0)
            for rr in range(1, RPC):
                nc.vector.memset(pz_b[:, 16 * rr:16 * rr + 1], 0.0)

            # ---- S matmuls: one N=512 band per half, corners per block
            # (each closing its block's accumulation group) ----
            def s_corner(b):
                sl = slice(b * 128, (b + 1) * 128)
                rhs = pz_b[:, 0:128] if b == 0 else p_b[:, (b - 1) * 128:b * 128]
                nc.tensor.matmul(s_ps[:, sl], corner, rhs,
                                 start=False, stop=True, skip_group_check=True)

            nc.tensor.matmul(s_ps[:, H1], band0, p_b[:, H1],
                             start=True, stop=False, skip_group_check=True)
            for b in (5, 6, 7):
                s_corner(b)
            nc.tensor.matmul(s_ps[:, H0], band0, p_b[:, H0],
                             start=True, stop=False, skip_group_check=True)
            for b in (0, 1, 2, 3, 4):
                s_corner(b)

            # ---- 1/S ----
            for h in (H0, H1):
                nc.vector.reciprocal_approx_fast(rcp_b[:, h], s_ps[:, h])
            # ---- R = emit * (1/S); h1 on the idle Pool engine so both
            # halves finish together and Z unblocks earlier ----
            nc.gpsimd.tensor_mul(r_b[:, H1], e_b[:, H1], rcp_b[:, H1])
            nc.vector.tensor_mul(r_b[:, H0], e_b[:, H0], rcp_b[:, H0])

            # masked wrap operand for Z block 7: rz[:, 1:129] streams
            # r_b[:, 1:128]+pad; row-start cols (j%16==0) and col 128 zero
            nc.scalar.copy(rz_b[:, 1:128], r_b[:, 1:128])
            nc.vector.memset(rz_b[:, 128:129], 0.0)
            for rr in range(1, RPC):
                nc.vector.memset(rz_b[:, 16 * rr:16 * rr + 1], 0.0)

            # ---- Z matmuls: one N=512 band per half, corners per block ----
            def z_corner(b):
                sl = slice(b * 128, (b + 1) * 128)
                rhs = (rz_b[:, 1:129] if b == NBLK - 1
                       else r_b[:, (b + 1) * 128:(b + 2) * 128])
                nc.tensor.matmul(z_ps[:, sl], cornera, rhs,
                                 start=False, stop=True, skip_group_check=True)

            nc.tensor.matmul(z_ps[:, H0], banda, r_b[:, H0],
                             start=True, stop=False, skip_group_check=True)
            for b in (0, 1, 2):
                z_corner(b)
            nc.tensor.matmul(z_ps[:, H1], banda, r_b[:, H1],
                             start=True, stop=False, skip_group_check=True)
            for b in (3, 4, 5, 6, 7):
                z_corner(b)

            # ---- out = P * Z (fp32), store directly in layout B ----
            nc.vector.tensor_mul(o_b[:, H0], p_b[:, H0], z_ps[:, H0])
            nc.vector.tensor_mul(o_b[:, H1], p_b[:, H1], z_ps[:, H1])
            nc.sync.dma_start(
                bass.AP(out_t, 0, [[NF, NPART], [1, 512]]), o_b[:, H0])
            nc.scalar.dma_start(
                bass.AP(out_t, 512, [[NF, NPART], [1, 512]]), o_b[:, H1])

    nc.compile()
    return nc


def make_in_maps(emit_probs, softmax_logits):
    lg = np.asarray(softmax_logits, dtype=np.float32)
    hi = lg.astype(np.float16)
    lo = (lg - hi.astype(np.float32)).astype(np.float16)
    em16 = np.asarray(emit_probs, dtype=np.float16)
    consts = make_consts()
    maps = []
    for k in range(NCORES):
        rows = slice(k * RPC, (k + 1) * RPC)
        maps.append({
            "lg_hi": _perm(hi[rows]),
            "lg_lo": _perm(lo[rows]),
            "em16": _perm(em16[rows]),
            "consts16": consts,
        })
    return maps


_NC_CACHE = None


def _get_nc():
    global _NC_CACHE
    if _NC_CACHE is None:
        _NC_CACHE = build_nc()
    return _NC_CACHE


def run(emit_probs, softmax_logits, trace=False, **kwargs):
    nc = _get_nc()
    in_maps = make_in_maps(emit_probs, softmax_logits)
    res = run_bass_kernel_spmd(
        nc, in_maps, core_ids=list(range(NCORES)), trace=trace, **kwargs
    )
    out = np.concatenate(
        [unperm_out(res.results[k]["out"]) for k in range(NCORES)], axis=0
    )
    return out, res


def kernel(emit_probs, softmax_logits):
    return run(emit_probs, softmax_logits)[0]



# revision 3
# speedup vs baseline: 1.1436x; 1.1436x over previous
"""MoChA stable chunkwise attention (window w=16) on 8 Trainium2 NeuronCores.

The reference's stabilizing moving-max cancels algebraically:
    P[t] = exp(logits[t]);  S[u] = sum_{v=u-15..u} P[v]
    R[u] = emit[u]/S[u];    out[t] = P[t] * sum_{k=0..15} R[t+k]
Both width-16 window sums run on the TensorEngine as banded matmuls in a
transposed layout (partition = t mod 128, free = (block, row, chunk)); the
cross-block window wrap is handled by corner matmuls accumulating in PSUM.
Corner matmuls are merged into wide-N passes; the sequence-boundary wrap
(block 0 of S, last block of Z) uses strided rhs/out access patterns that
skip the row-start/row-end columns, so no masked scratch copies are needed.
Logits/emit travel as fp16 (exp output is fp16 anyway), the output returns
as fp16 and is upcast on the host. Junk matmuls at kernel start warm the
PE HAM clock gate (1.2 -> 2.4 GHz) while the input DMAs are in flight.

Self-contained: only numpy + concourse (on PYTHONPATH) required.
"""

import numpy as np

import concourse.bass as bass
import concourse.tile as tile
import concourse.mybir as mybir
from concourse import bacc
from concourse.bass_utils import run_bass_kernel_spmd

F32 = mybir.dt.float32
F16 = mybir.dt.float16
ACTF = mybir.ActivationFunctionType

B, T = 64, 16384
NCORES = 8
RPC = B // NCORES        # 8 rows/core
NCH = 16                 # chunks per row
NPART = 128
NBLK = 8                 # blocks per chunk
W = 16                   # window
NF = RPC * T // 128      # 1024 layout-B columns
H0 = slice(0, 512)
H1 = slice(512, 1024)

N_WARM_PRE = 6           # junk matmuls (N=512) before the S matmuls
N_WARM_MID = 2           # junk matmuls between S and Z groups


def make_consts():
    k = np.arange(128)[:, None]
    m = np.arange(128)[None, :]
    band0 = (m - k >= 0) & (m - k <= W - 1)            # S within-block
    corner = (k - m >= 128 - W + 1) & (k - m <= 127)   # S from prev block
    banda = (k - m >= 0) & (k - m <= W - 1)            # Z within-block
    cornera = (m - k >= 128 - W + 1) & (m - k <= 127)  # Z from next block
    return np.concatenate(
        [x.astype(np.float16) for x in (band0, corner, banda, cornera)],
        axis=1,
    )  # [128, 512]


def _perm(a):
    """[RPC, T] -> layout B [128, NF]: full host-side transpose, so device
    loads are plain contiguous [128 partitions x NF] DMAs (no xbar)."""
    return np.ascontiguousarray(
        a.reshape(RPC, NCH, NBLK, 128).transpose(3, 2, 0, 1).reshape(128, NF)
    )


def unperm_out(o):
    """[128, NF] layout B -> [RPC, T]."""
    return np.ascontiguousarray(
        o.reshape(128, NBLK, RPC, NCH)
        .transpose(2, 3, 1, 0)
        .reshape(RPC, T)
    )


def _wrap_out(ps_block):
    """[128,128] PSUM block view -> strided [128,8,15] skipping c=0 cols,
    offset by one chunk column."""
    return ps_block.rearrange("p (g c) -> p g c", c=16)[:, :, 1:16]


def _wrap_rhs(sb_block):
    """[128,128] SBUF block view -> strided [128,8,15] over c=0..14."""
    return sb_block.rearrange("p (g c) -> p g c", c=16)[:, :, 0:15]


def build_nc():
    nc = bacc.Bacc("TRN2", target_bir_lowering=False, debug=False,
                   num_devices=NCORES)
    lg_t = nc.dram_tensor("lg16", [NPART, NF], F16, kind="ExternalInput")
    em_t = nc.dram_tensor("em16", [NPART, NF], F16, kind="ExternalInput")
    kc_t = nc.dram_tensor("consts16", [NPART, 512], F16, kind="ExternalInput")
    out_t = nc.dram_tensor("out", [NPART, NF], F16, kind="ExternalOutput")

    with tile.TileContext(nc) as tc:
        with (
            tc.tile_pool(name="sb", bufs=1) as sb,
            tc.tile_pool(name="ps", bufs=1, space="PSUM") as ps,
        ):
            kb = sb.tile([NPART, 512], F16, tag="kb")
            warm = sb.tile([NPART, 128], F16, tag="warm")
            lg_b = sb.tile([NPART, NF], F16, tag="lg_b")
            e_b = sb.tile([NPART, NF], F16, tag="e_b")
            p_b = sb.tile([NPART, NF], F16, tag="p_b")
            rcp_b = sb.tile([NPART, NF], F32, tag="rcp_b")
            r_b = sb.tile([NPART, NF], F16, tag="r_b")
            o_b = sb.tile([NPART, NF], F16, tag="o_b")

            s_ps = ps.tile([NPART, NF], F32, tag="s")
            z_ps = ps.tile([NPART, NF], F32, tag="z")
            w_ps = ps.tile([NPART, 512], F32, tag="w")

            band0 = kb[:, 0:128]
            corner = kb[:, 128:256]
            banda = kb[:, 256:384]
            cornera = kb[:, 384:512]

            # ---- warmup weight (Pool is idle early) ----
            nc.gpsimd.memset(warm[:, :], 1.0)

            # ---- loads: logits halves first (exp gates everything),
            # emit later (needed only by Rmul), consts on the ACT ring ----
            nc.sync.dma_start(
                lg_b[:, H0], bass.AP(lg_t, 0, [[NF, NPART], [1, 512]]))
            nc.sync.dma_start(
                lg_b[:, H1], bass.AP(lg_t, 512, [[NF, NPART], [1, 512]]))
            nc.scalar.dma_start(
                kb[:, :], bass.AP(kc_t, 0, [[512, NPART], [1, 512]]))
            nc.sync.dma_start(
                e_b[:, :], bass.AP(em_t, 0, [[NF, NPART], [1, NF]]))

            # ---- PE warmup: junk matmuls keep the HAM activity window busy
            # while DMAs land, so the real matmuls run at 2.4 GHz ----
            warm_rhs = warm[:, :].unsqueeze(1).broadcast_to([NPART, 4, 128])
            for _ in range(N_WARM_PRE):
                nc.tensor.matmul(w_ps[:, :], warm[:, :], warm_rhs,
                                 start=True, stop=True, skip_group_check=True)

            # ---- P = exp(logits), fp16 ----
            nc.scalar.activation(p_b[:, H0], lg_b[:, H0], ACTF.Exp)
            nc.scalar.activation(p_b[:, H1], lg_b[:, H1], ACTF.Exp)

            # ---- S matmuls (PSUM bank groups H0/H1) ----
            nc.tensor.matmul(s_ps[:, H0], band0, p_b[:, H0],
                             start=True, stop=False, skip_group_check=True)
            nc.tensor.matmul(s_ps[:, 128:512], corner, p_b[:, 0:384],
                             start=False, stop=False, skip_group_check=True)
            # sequence-wrap corner for block 0: out cols (g,c>=1) from the
            # tail of block 7 one chunk earlier; row-start cols skipped
            nc.tensor.matmul(_wrap_out(s_ps[:, 0:128]), corner,
                             _wrap_rhs(p_b[:, 896:1024]),
                             start=False, stop=True, skip_group_check=True)
            nc.tensor.matmul(s_ps[:, H1], band0, p_b[:, H1],
                             start=True, stop=False, skip_group_check=True)
            nc.tensor.matmul(s_ps[:, H1], corner, p_b[:, 384:896],
                             start=False, stop=True, skip_group_check=True)

            # ---- 1/S on DVE; R = emit * (1/S) split DVE/Pool ----
            nc.vector.reciprocal_approx_fast(rcp_b[:, H0], s_ps[:, H0])
            nc.gpsimd.tensor_mul(r_b[:, H0], e_b[:, H0], rcp_b[:, H0])
            nc.vector.reciprocal_approx_fast(rcp_b[:, H1], s_ps[:, H1])
            nc.vector.tensor_mul(r_b[:, H1], e_b[:, H1], rcp_b[:, H1])

            # mid warmup keeps PE busy while Rmul runs
            for _ in range(N_WARM_MID):
                nc.tensor.matmul(w_ps[:, :], warm[:, :], warm_rhs,
                                 start=True, stop=True, skip_group_check=True)

            # ---- Z matmuls ----
            nc.tensor.matmul(z_ps[:, H0], banda, r_b[:, H0],
                             start=True, stop=False, skip_group_check=True)
            nc.tensor.matmul(z_ps[:, H0], cornera, r_b[:, 128:640],
                             start=False, stop=True, skip_group_check=True)
            nc.tensor.matmul(z_ps[:, H1], banda, r_b[:, H1],
                             start=True, stop=False, skip_group_check=True)
            nc.tensor.matmul(z_ps[:, 512:896], cornera, r_b[:, 640:1024],
                             start=False, stop=False, skip_group_check=True)
            # sequence-wrap corner for the last block: out cols c<=14 from
            # block 0 one chunk later; row-end cols skipped
            nc.tensor.matmul(
                z_ps[:, 896:1024].rearrange("p (g c) -> p g c", c=16)[:, :, 0:15],
                cornera,
                p_wrap_z := r_b[:, 0:128].rearrange("p (g c) -> p g c", c=16)[:, :, 1:16],
                start=False, stop=True, skip_group_check=True)

            # ---- out = P * Z (fp16), store halves as they finish ----
            nc.vector.tensor_mul(o_b[:, H0], p_b[:, H0], z_ps[:, H0])
            nc.sync.dma_start(
                bass.AP(out_t, 0, [[NF, NPART], [1, 512]]), o_b[:, H0])
            nc.vector.tensor_mul(o_b[:, H1], p_b[:, H1], z_ps[:, H1])
            nc.sync.dma_start(
                bass.AP(out_t, 512, [[NF, NPART], [1, 512]]), o_b[:, H1])

    nc.compile()
    return nc


def make_in_maps(emit_probs, softmax_logits):
    lg16 = np.asarray(softmax_logits, dtype=np.float16)
    em16 = np.asarray(emit_probs, dtype=np.float16)
    consts = make_consts()
    maps = []
    for c in range(NCORES):
        rows = slice(c * RPC, (c + 1) * RPC)
        maps.append({
            "lg16": _perm(lg16[rows]),
            "em16": _perm(em16[rows]),
            "consts16": consts,
        })
    return maps


_NC_CACHE = None


def _get_nc():
    global _NC_CACHE
    if _NC_CACHE is None:
        _NC_CACHE = build_nc()
    return _NC_CACHE


def run(emit_probs, softmax_logits, trace=False, **kwargs):
    nc = _get_nc()
    in_maps = make_in_maps(emit_probs, softmax_logits)
    res = run_bass_kernel_spmd(
        nc, in_maps, core_ids=list(range(NCORES)), trace=trace, **kwargs
    )
    out = np.concatenate(
        [unperm_out(res.results[c]["out"]).astype(np.float32)
         for c in range(NCORES)],
        axis=0,
    )
    return out, res


def kernel(emit_probs, softmax_logits):
    return run(emit_probs, softmax_logits)[0]
